# revision 4
# baseline (speedup 1.0000x reference)
"""Trainium2 Bass kernel for GaussianKernelConv.

Math: out[b,n,p] = mean_k exp(-||x[b,n,k,:] - kp[p,:]||^2 / (2 sigma^2))

Per-core dataflow (one batch b of N=8192, K=32, P=16):
  MM1 (TensorE): t = FS * v_mm, where v_mm = x.(kp_p/s^2) - ||x||^2/(2 s^2).
    Output partitions = (k_j in 8, p in 16). Four row-group matmuls
    (tile_position (32m,0), contract 32 = (k_j, [x0,x1,x2,||x||^2])) per
    512-n superpass; row group m computes the k''=m slice (k = 4*k_j + m)
    for all 512 n, into psum bank m ([128, 512] cols = n_local).
  exp (split):
    ScalarE: exact exp via activation(Exp, scale=1/FS, bias=bias_p) on psum
      cols [0, XSPLIT).
    VectorE: custom 8-stage DVE op  [ (t + C0_p)^2 + FB ]^32  ~= e^(v_mm+bias_p)
      on cols [XSPLIT, 2048)  (max rel err ~9e-3 on the relevant range,
      monotone-tiny below; C0_p = FA + FS*bias_p per partition). Only ~30%
      of each output's K terms take this path, diluting its error ~3x.
  MM2 (TensorE): K-reduction. Contract the 128 (k_j,p) partitions with a
    [128,16] p-selector (1/K); 4 accumulating 512-col matmuls fold k''.
    Result [16, 512] is written into the just-freed psum bank 0, cast
    fp16 -> SBUF (VectorE) -> DMA out.
  PE emission is software-pipelined: MM1(sp+1) is emitted before MM2(sp) so
  the TensorE never head-of-line blocks on the exp engines.

Sharding: data-parallel over batch B=8 -> 8 cores, one batch each.
"""

import sys

for _p in ("/opt/trn_rl_repo",):
    if _p not in sys.path:
        sys.path.insert(0, _p)

import numpy as np

B, N, K, C, P = 8, 8192, 32, 3, 16
NSP = 16          # superpasses per core; each covers 512 n
NCH = 4           # DMA chunks (epochs); 4 superpasses each
XSPLIT = 1408     # psum cols [0,XSPLIT) -> ScalarE exp; rest -> DVE custom op

# exp approximation constants: [ (FS*w + FA)^2 + FB ]^32 ~= e^w on w in [-12, 0]
FS = 0.02010519997941581
FA = 0.7677708409964104
FB = 0.41026898832429365

_CACHE = {}


def _register_dve_op():
    """Register the 8-stage [quad]^32 exp op with concourse's custom-DVE table."""
    from concourse import dve_ops as dvo
    from concourse.dve_spec import Spec, Src0, C0, C1, sq, lower
    from concourse.dve_uop import DveOpSpec

    name = "EXP_POW32_ANT"
    if name in dvo._SUB_OPCODE_FOR_NAME:
        for op in dvo.OPS:
            if op.name == name:
                return op

    def _ref(in0, in1, s0, s1, imm2):
        t = in0.astype(np.float32) + np.asarray(s0, np.float32).reshape(-1, 1)
        u = t * t + np.float32(s1)
        for _ in range(5):
            u = u * u
        return u

    body = sq(Src0 + C0) + C1
    for _ in range(5):
        body = sq(body)
    spec = Spec(body=body, reference=_ref)
    row = dvo._CUSTOM_DVE_ROW_BASE + len(dvo.OPS)
    shas = {
        ver: DveOpSpec(name=name, opcode=row, uops=lower(spec, ver=ver),
                       rd1_en=False).sha(ver)
        for ver in ("v3", "v4")
    }
    op = dvo.DveOp(name, spec, subdim=False, uops_sha=shas)
    dvo.OPS.append(op)
    dvo.CUSTOM_DVE_SPECS[name] = spec
    dvo._SUB_OPCODE_FOR_NAME[name] = row
    return op


def _build_nc():
    from concourse import bacc, mybir
    from concourse.tile import TileContext

    exp_op = _register_dve_op()
    f16, f32 = mybir.dt.float16, mybir.dt.float32
    Act = mybir.ActivationFunctionType

    nc = bacc.Bacc(None, target_bir_lowering=False)
    xin = nc.declare_dram_parameter("xin", [NCH, 128, 2048], f16, isOutput=False)
    w1 = nc.declare_dram_parameter("w1", [128, 128], f16, isOutput=False)
    w2 = nc.declare_dram_parameter("w2", [128, 16], f16, isOutput=False)
    c0 = nc.declare_dram_parameter("c0", [128, 1], f32, isOutput=False)
    ab = nc.declare_dram_parameter("ab", [128, 1], f32, isOutput=False)
    out = nc.declare_dram_parameter("out", [NCH, 16, 2048], f16, isOutput=True)

    act_scale = float(1.0 / FS)

    with TileContext(nc) as tc:
        with (
            tc.tile_pool(name="const", bufs=1) as cpool,
            tc.tile_pool(name="xp", bufs=2) as xpool,
            tc.tile_pool(name="ep", bufs=3) as epool,
            tc.tile_pool(name="ob", bufs=2) as opool,
            tc.tile_pool(name="ps", bufs=2, space="PSUM") as ppool,
        ):
            w1_t = cpool.tile([128, 128], f16, tag="w1")
            nc.sync.dma_start(out=w1_t[:], in_=w1[:])
            w2_t = cpool.tile([128, 16], f16, tag="w2")
            nc.sync.dma_start(out=w2_t[:], in_=w2[:])
            c0_t = cpool.tile([128, 1], f32, tag="c0")
            nc.sync.dma_start(out=c0_t[:], in_=c0[:])
            ab_t = cpool.tile([128, 1], f32, tag="ab")
            nc.sync.dma_start(out=ab_t[:], in_=ab[:])

            xts, ots = {}, {}
            pend = []    # (pt, et, ch, q) awaiting MM2+copy, one sp behind

            def emit_mm2(pt, et, ch, q):
                for i in range(4):
                    nc.tensor.matmul(
                        pt[0:16, 0:512],
                        w2_t[:, 0:16],
                        et[:, i * 512:(i + 1) * 512],
                        start=(i == 0), stop=(i == 3),
                    )
                nc.vector.tensor_copy(ots[ch][:, q * 512:(q + 1) * 512],
                                      pt[0:16, 0:512])
                if q == 3:
                    nc.sync.dma_start(out=out[ch], in_=ots[ch][:])

            for sp in range(NSP):
                ch, q = sp // 4, sp % 4
                if q == 0:
                    xt = xpool.tile([128, 2048], f16, tag="x")
                    xts[ch] = xt
                    nc.sync.dma_start(out=xt[:], in_=xin[ch])
                    ot = opool.tile([16, 2048], f16, tag="o")
                    ots[ch] = ot
                pt = ppool.tile([128, 2048], f32, tag="arg")
                rhs = xts[ch][:, q * 512:(q + 1) * 512]
                for m in range(4):
                    nc.tensor.matmul(
                        pt[:, m * 512:(m + 1) * 512],
                        w1_t[m * 32:(m + 1) * 32, :],
                        rhs[m * 32:(m + 1) * 32, :],
                        start=True, stop=True, tile_position=(m * 32, 0),
                    )
                et = epool.tile([128, 2048], f16, tag="e")
                nc.scalar.activation(
                    et[:, 0:XSPLIT], pt[:, 0:XSPLIT], Act.Exp,
                    bias=ab_t[:], scale=act_scale,
                )
                nc.vector._custom_dve(
                    exp_op, out=et[:, XSPLIT:2048], in0=pt[:, XSPLIT:2048],
                    s0=c0_t[:], s1=float(FB),
                )
                pend.append((pt, et, ch, q))
                if len(pend) > 1:
                    emit_mm2(*pend.pop(0))
            emit_mm2(*pend.pop(0))

    nc.finalize()
    return nc


def _host_pack(x):
    """x: (B, N, K, C) fp32 -> (B, NCH, 128, 2048) fp16 rhs layout.

    partition = 32*m + 4*k_j + ct  (ct in 0..3: x0,x1,x2,||x||^2; m = k'')
    col       = (sp%4)*512 + n_local
    with n = 512*sp + n_local, k = 4*k_j + m.
    """
    xr = x.reshape(B, NSP, 512, 8, 4, C)             # b,sp,nl,kj,m,c
    s = (xr.astype(np.float64) ** 2).sum(-1)         # b,sp,nl,kj,m
    x4 = np.empty((B, NSP, 512, 8, 4, 4), dtype=np.float16)
    x4[..., :C] = xr.astype(np.float16)
    x4[..., C] = s.astype(np.float16)
    # -> (b, sp, m, kj, ct, nl)
    d = x4.transpose(0, 1, 4, 3, 5, 2)
    d = np.ascontiguousarray(d.reshape(B, NSP, 128, 512))
    return np.ascontiguousarray(
        d.reshape(B, NCH, 4, 128, 512).transpose(0, 1, 3, 2, 4)
    ).reshape(B, NCH, 128, 2048)


def _host_weights(kernel_points, sigma):
    kp = np.asarray(kernel_points, dtype=np.float64)
    s2 = float(sigma) ** 2
    bias = -(kp ** 2).sum(-1) / (2.0 * s2)           # (P,)

    w1 = np.zeros((128, 128), dtype=np.float16)
    wv = np.zeros((4, P), dtype=np.float64)
    wv[:C] = (FS * kp / s2).T
    wv[C] = -FS / (2.0 * s2)
    for m in range(4):
        for kj in range(8):
            for ct in range(4):
                w1[32 * m + 4 * kj + ct, 16 * kj:16 * (kj + 1)] = \
                    wv[ct].astype(np.float16)

    w2 = np.zeros((128, 16), dtype=np.float16)
    for kj in range(8):
        for p in range(P):
            w2[16 * kj + p, p] = np.float16(1.0 / K)

    c0 = np.zeros((128, 1), dtype=np.float32)
    ab = np.zeros((128, 1), dtype=np.float32)
    for kj in range(8):
        for p in range(P):
            c0[16 * kj + p, 0] = FA + FS * bias[p]
            ab[16 * kj + p, 0] = bias[p]
    return w1, w2, c0, ab


def _host_unpack(outs):
    """outs: list of 8 per-core (NCH, 16, 2048) fp16 -> (B, N, P) fp32."""
    res = np.empty((B, N, P), dtype=np.float32)
    for b, o in enumerate(outs):
        # out[ch, p, q*512 + nl] = res[n = (4ch+q)*512 + nl, p]
        r = o.reshape(NCH, P, 4, 512).transpose(0, 2, 3, 1)
        res[b] = r.reshape(N, P).astype(np.float32)
    return res


def _run(inputs, trace=False, tmpdir=None, trace_cores=None):
    from concourse.bass_utils import run_bass_kernel_spmd

    x = np.asarray(inputs["neighborhoods"], dtype=np.float32)
    d = _host_pack(x)
    w1, w2, c0, ab = _host_weights(inputs["kernel_points"], inputs["sigma"])

    if "nc" not in _CACHE:
        _CACHE["nc"] = _build_nc()
    nc = _CACHE["nc"]

    core_ids = list(range(B))
    in_maps = [
        {"xin": d[b], "w1": w1, "w2": w2, "c0": c0, "ab": ab}
        for b in range(B)
    ]
    res = run_bass_kernel_spmd(nc, in_maps, core_ids, trace=trace,
                               tmpdir=tmpdir, trace_cores=trace_cores)
    return _host_unpack([res.results[b]["out"] for b in range(B)]), res


def kernel(neighborhoods, kernel_points, sigma):
    out, _ = _run({"neighborhoods": neighborhoods,
                   "kernel_points": kernel_points, "sigma": sigma})
    return out


# revision 7
# speedup vs baseline: 1.1773x; 1.1773x over previous
"""Trainium2 Bass kernel for GaussianKernelConv.

Math: out[b,n,p] = mean_k exp(-||x[b,n,k,:] - kp[p,:]||^2 / (2 sigma^2))

Per-core dataflow (one batch b of N=8192, K=32, P=16):
  MM1 (TensorE): t = FS * v_mm, where v_mm = x.(kp_p/s^2) - ||x||^2/(2 s^2).
    Output partitions = (k_j in 8, p in 16). Four row-group matmuls
    (tile_position (32m,0), contract 32 = (k_j, [x0,x1,x2,||x||^2])) per
    512-n superpass; row group m computes the k''=m slice (k = 4*k_j + m)
    for all 512 n, into psum bank m ([128, 512] cols = n_local).
  exp (split):
    ScalarE: exact exp via activation(Exp, scale=1/FS, bias=bias_p) on psum
      cols [0, XSPLIT).
    VectorE: custom 8-stage DVE op  [ (t + C0_p)^2 + FB ]^32  ~= e^(v_mm+bias_p)
      on cols [XSPLIT, 2048)  (max rel err ~9e-3 on the relevant range,
      monotone-tiny below; C0_p = FA + FS*bias_p per partition). Only ~30%
      of each output's K terms take this path, diluting its error ~3x.
  MM2 (TensorE): K-reduction. Contract the 128 (k_j,p) partitions with a
    [128,16] p-selector (1/K). Four column-tiled matmuls (tile_position
    (0,32m), one per n-quarter) each stream 512 cols with a stride-0
    broadcast out AP that revisits the same 128 psum cols 4x -- the k''
    fold happens via PSUM has_written accumulation inside one matmul.
    Result [(m,p'), n'] lands in the just-freed psum bank 0, cast
    fp16 -> SBUF (VectorE) -> DMA out. A 12-matmul warmup burst under the
    first DMA brings the PE HAM clock gate to 8/8 before the pipeline.
  PE emission is software-pipelined: MM1(sp+1) is emitted before MM2(sp) so
  the TensorE never head-of-line blocks on the exp engines.

Sharding: data-parallel over batch B=8 -> 8 cores, one batch each.
"""

import sys

for _p in ("/opt/trn_rl_repo",):
    if _p not in sys.path:
        sys.path.insert(0, _p)

import numpy as np

B, N, K, C, P = 8, 8192, 32, 3, 16
NSP = 16          # superpasses per core; each covers 512 n
NCH = 4           # DMA chunks (epochs); 4 superpasses each
XSPLIT = 1184     # psum cols [0,XSPLIT) -> ScalarE exp; rest -> DVE custom op

# exp approximation constants: [ (FS*w + FA)^2 + FB ]^32 ~= e^w on w in [-12, 0]
FS = 0.02010519997941581
FA = 0.7677708409964104
FB = 0.41026898832429365

_CACHE = {}


def _register_dve_op():
    """Register the 8-stage [quad]^32 exp op with concourse's custom-DVE table."""
    from concourse import dve_ops as dvo
    from concourse.dve_spec import Spec, Src0, C0, C1, sq, lower
    from concourse.dve_uop import DveOpSpec

    name = "EXP_POW32_ANT"
    if name in dvo._SUB_OPCODE_FOR_NAME:
        for op in dvo.OPS:
            if op.name == name:
                return op

    def _ref(in0, in1, s0, s1, imm2):
        t = in0.astype(np.float32) + np.asarray(s0, np.float32).reshape(-1, 1)
        u = t * t + np.float32(s1)
        for _ in range(5):
            u = u * u
        return u

    body = sq(Src0 + C0) + C1
    for _ in range(5):
        body = sq(body)
    spec = Spec(body=body, reference=_ref)
    row = dvo._CUSTOM_DVE_ROW_BASE + len(dvo.OPS)
    shas = {
        ver: DveOpSpec(name=name, opcode=row, uops=lower(spec, ver=ver),
                       rd1_en=False).sha(ver)
        for ver in ("v3", "v4")
    }
    op = dvo.DveOp(name, spec, subdim=False, uops_sha=shas)
    dvo.OPS.append(op)
    dvo.CUSTOM_DVE_SPECS[name] = spec
    dvo._SUB_OPCODE_FOR_NAME[name] = row
    return op


def _build_nc():
    from concourse import bacc, mybir
    from concourse.tile import TileContext

    exp_op = _register_dve_op()
    f16, f32 = mybir.dt.float16, mybir.dt.float32
    Act = mybir.ActivationFunctionType

    nc = bacc.Bacc(None, target_bir_lowering=False)
    xin = nc.declare_dram_parameter("xin", [NCH, 128, 2048], f16, isOutput=False)
    w1 = nc.declare_dram_parameter("w1", [128, 128], f16, isOutput=False)
    w2 = nc.declare_dram_parameter("w2", [128, 16], f16, isOutput=False)
    c0 = nc.declare_dram_parameter("c0", [128, 1], f32, isOutput=False)
    ab = nc.declare_dram_parameter("ab", [128, 1], f32, isOutput=False)
    out = nc.declare_dram_parameter("out", [NCH, 128, 512], f16, isOutput=True)

    act_scale = float(1.0 / FS)

    with TileContext(nc) as tc:
        with (
            tc.tile_pool(name="const", bufs=1) as cpool,
            tc.tile_pool(name="xp", bufs=2) as xpool,
            tc.tile_pool(name="ep", bufs=3) as epool,
            tc.tile_pool(name="ob", bufs=2) as opool,
            tc.tile_pool(name="ps", bufs=2, space="PSUM") as ppool,
        ):
            w1_t = cpool.tile([128, 128], f16, tag="w1")
            nc.sync.dma_start(out=w1_t[:], in_=w1[:])
            w2_t = cpool.tile([128, 16], f16, tag="w2")
            nc.sync.dma_start(out=w2_t[:], in_=w2[:])
            c0_t = cpool.tile([128, 1], f32, tag="c0")
            nc.sync.dma_start(out=c0_t[:], in_=c0[:])
            ab_t = cpool.tile([128, 1], f32, tag="ab")
            nc.sync.dma_start(out=ab_t[:], in_=ab[:])

            xts, ots = {}, {}
            pend = []    # (pt, et, ch, q) awaiting MM2+copy, one sp behind

            def emit_mm2(pt, et, ch, q):
                rhs4 = et[:].rearrange("z (i mm n) -> z i mm n", i=4, mm=4)
                for m in range(4):
                    ob = pt[32 * m:32 * m + 16, 0:128].rearrange(
                        "p (i n) -> p i n", i=1).broadcast_to([16, 4, 128])
                    nc.tensor.matmul(
                        ob, w2_t[:, 0:16], rhs4[:, :, m, :],
                        start=True, stop=True, tile_position=(0, 32 * m),
                    )
                nc.vector.tensor_copy(ots[ch][:, q * 128:(q + 1) * 128],
                                      pt[:, 0:128])
                if q == 3:
                    nc.sync.dma_start(out=out[ch], in_=ots[ch][:])

            for sp in range(NSP):
                ch, q = sp // 4, sp % 4
                if q == 0:
                    xt = xpool.tile([128, 2048], f16, tag="x")
                    xts[ch] = xt
                    nc.sync.dma_start(out=xt[:], in_=xin[ch])
                    ot = opool.tile([128, 512], f16, tag="o")
                    ots[ch] = ot
                pt = ppool.tile([128, 2048], f32, tag="arg")
                if sp == 0:
                    # PE warmup under the first DMA: ~12 matmuls bring the
                    # HAM clock gate to 8/8 before the pipeline starts.
                    for _ in range(12):
                        nc.tensor.matmul(pt[:, 0:128], w1_t[0:32, :],
                                         w1_t[0:32, :], start=True, stop=True,
                                         tile_position=(0, 0))
                rhs = xts[ch][:, q * 512:(q + 1) * 512]
                for m in range(4):
                    nc.tensor.matmul(
                        pt[:, m * 512:(m + 1) * 512],
                        w1_t[m * 32:(m + 1) * 32, :],
                        rhs[m * 32:(m + 1) * 32, :],
                        start=True, stop=True, tile_position=(m * 32, 0),
                    )
                et = epool.tile([128, 2048], f16, tag="e")
                nc.scalar.activation(
                    et[:, 0:XSPLIT], pt[:, 0:XSPLIT], Act.Exp,
                    bias=ab_t[:], scale=act_scale,
                )
                nc.vector._custom_dve(
                    exp_op, out=et[:, XSPLIT:2048], in0=pt[:, XSPLIT:2048],
                    s0=c0_t[:], s1=float(FB),
                )
                pend.append((pt, et, ch, q))
                if len(pend) > 1:
                    emit_mm2(*pend.pop(0))
            emit_mm2(*pend.pop(0))

    nc.finalize()
    return nc


def _host_pack(x):
    """x: (B, N, K, C) fp32 -> (B, NCH, 128, 2048) fp16 rhs layout.

    partition = 32*m + 4*k_j + ct  (ct in 0..3: x0,x1,x2,||x||^2; m = k'')
    col       = (sp%4)*512 + n_local
    with n = 512*sp + n_local, k = 4*k_j + m.
    """
    xr = x.reshape(B, NSP, 512, 8, 4, C)             # b,sp,nl,kj,m,c
    s = (xr.astype(np.float64) ** 2).sum(-1)         # b,sp,nl,kj,m
    x4 = np.empty((B, NSP, 512, 8, 4, 4), dtype=np.float16)
    x4[..., :C] = xr.astype(np.float16)
    x4[..., C] = s.astype(np.float16)
    # -> (b, sp, m, kj, ct, nl)
    d = x4.transpose(0, 1, 4, 3, 5, 2)
    d = np.ascontiguousarray(d.reshape(B, NSP, 128, 512))
    return np.ascontiguousarray(
        d.reshape(B, NCH, 4, 128, 512).transpose(0, 1, 3, 2, 4)
    ).reshape(B, NCH, 128, 2048)


def _host_weights(kernel_points, sigma):
    kp = np.asarray(kernel_points, dtype=np.float64)
    s2 = float(sigma) ** 2
    bias = -(kp ** 2).sum(-1) / (2.0 * s2)           # (P,)

    w1 = np.zeros((128, 128), dtype=np.float16)
    wv = np.zeros((4, P), dtype=np.float64)
    wv[:C] = (FS * kp / s2).T
    wv[C] = -FS / (2.0 * s2)
    for m in range(4):
        for kj in range(8):
            for ct in range(4):
                w1[32 * m + 4 * kj + ct, 16 * kj:16 * (kj + 1)] = \
                    wv[ct].astype(np.float16)

    w2 = np.zeros((128, 16), dtype=np.float16)
    for kj in range(8):
        for p in range(P):
            w2[16 * kj + p, p] = np.float16(1.0 / K)

    c0 = np.zeros((128, 1), dtype=np.float32)
    ab = np.zeros((128, 1), dtype=np.float32)
    for kj in range(8):
        for p in range(P):
            c0[16 * kj + p, 0] = FA + FS * bias[p]
            ab[16 * kj + p, 0] = bias[p]
    return w1, w2, c0, ab


def _host_unpack(outs):
    """outs: list of 8 per-core (NCH, 128, 512) fp16 -> (B, N, P) fp32."""
    res = np.empty((B, N, P), dtype=np.float32)
    for b, o in enumerate(outs):
        # out[ch, 32m+p (p<16), q*128+n'] = res[n = (4ch+q)*512 + 128m + n', p]
        o4 = o.reshape(NCH, 4, 32, 4, 128)           # ch, m, part32, q, n'
        r = o4[:, :, :P, :, :].transpose(0, 3, 1, 4, 2)  # ch,q,m,n',p
        res[b] = r.reshape(N, P).astype(np.float32)
    return res


def _run(inputs, trace=False, tmpdir=None, trace_cores=None):
    from concourse.bass_utils import run_bass_kernel_spmd

    x = np.asarray(inputs["neighborhoods"], dtype=np.float32)
    d = _host_pack(x)
    w1, w2, c0, ab = _host_weights(inputs["kernel_points"], inputs["sigma"])

    if "nc" not in _CACHE:
        _CACHE["nc"] = _build_nc()
    nc = _CACHE["nc"]

    core_ids = list(range(B))
    in_maps = [
        {"xin": d[b], "w1": w1, "w2": w2, "c0": c0, "ab": ab}
        for b in range(B)
    ]
    res = run_bass_kernel_spmd(nc, in_maps, core_ids, trace=trace,
                               tmpdir=tmpdir, trace_cores=trace_cores)
    return _host_unpack([res.results[b]["out"] for b in range(B)]), res


def kernel(neighborhoods, kernel_points, sigma):
    out, _ = _run({"neighborhoods": neighborhoods,
                   "kernel_points": kernel_points, "sigma": sigma})
    return out


# revision 8
# speedup vs baseline: 1.2437x; 1.0563x over previous
"""Trainium2 Bass kernel for GaussianKernelConv.

Math: out[b,n,p] = mean_k exp(-||x[b,n,k,:] - kp[p,:]||^2 / (2 sigma^2))

Per-core dataflow (one batch b of N=8192, K=32, P=16):
  MM1 (TensorE): t = FS * v_mm, where v_mm = x.(kp_p/s^2) - ||x||^2/(2 s^2).
    Output partitions = (k_j in 8, p in 16). Four row-group matmuls
    (tile_position (32m,0), contract 32 = (k_j, [x0,x1,x2,||x||^2])) per
    512-n superpass; row group m computes the k''=m slice (k = 4*k_j + m)
    for all 512 n, into psum bank m ([128, 512] cols = n_local).
  exp (split):
    ScalarE: exact exp via activation(Exp, scale=1/FS, bias=bias_p) on psum
      cols [0, XSPLIT).
    VectorE: custom 8-stage DVE op  [ (t + C0_p)^2 + FB ]^32  ~= e^(v_mm+bias_p)
      on cols [XSPLIT, 2048)  (max rel err ~9e-3 on the relevant range,
      monotone-tiny below; C0_p = FA + FS*bias_p per partition). Only ~30%
      of each output's K terms take this path, diluting its error ~3x.
  MM2 (TensorE): K-reduction. Contract the 128 (k_j,p) partitions with a
    [128,16] p-selector (1/K). Four column-tiled matmuls (tile_position
    (0,32m), one per n-quarter) each stream 512 cols with a stride-0
    broadcast out AP that revisits the same 128 psum cols 4x -- the k''
    fold happens via PSUM has_written accumulation inside one matmul.
    Result [(m,p'), n'] lands in the just-freed psum bank 0, cast
    fp16 -> SBUF (VectorE) -> DMA out. A 12-matmul warmup burst under the
    first DMA brings the PE HAM clock gate to 8/8 before the pipeline.
  PE emission is software-pipelined: MM1(sp+1) is emitted before MM2(sp) so
  the TensorE never head-of-line blocks on the exp engines.

Sharding: data-parallel over batch B=8 -> 8 cores, one batch each.
"""

import sys

for _p in ("/opt/trn_rl_repo",):
    if _p not in sys.path:
        sys.path.insert(0, _p)

import numpy as np

B, N, K, C, P = 8, 8192, 32, 3, 16
NSP = 16          # superpasses per core; each covers 512 n
NCH = 4           # DMA chunks (epochs); 4 superpasses each
XSPLIT = 1024     # psum cols [0,XSPLIT) -> ScalarE exp; rest -> DVE custom op

# exp approximation constants: [ (FS*w + FA)^2 + FB ]^32 ~= e^w on w in [-12, 0]
FS = 0.02010519997941581
FA = 0.7677708409964104
FB = 0.41026898832429365

_CACHE = {}


def _register_dve_op():
    """Register the 8-stage [quad]^32 exp op with concourse's custom-DVE table."""
    from concourse import dve_ops as dvo
    from concourse.dve_spec import Spec, Src0, C0, C1, sq, lower
    from concourse.dve_uop import DveOpSpec

    name = "EXP_POW32_ANT"
    if name in dvo._SUB_OPCODE_FOR_NAME:
        for op in dvo.OPS:
            if op.name == name:
                return op

    def _ref(in0, in1, s0, s1, imm2):
        t = in0.astype(np.float32) + np.asarray(s0, np.float32).reshape(-1, 1)
        u = t * t + np.float32(s1)
        for _ in range(5):
            u = u * u
        return u

    body = sq(Src0 + C0) + C1
    for _ in range(5):
        body = sq(body)
    spec = Spec(body=body, reference=_ref)
    row = dvo._CUSTOM_DVE_ROW_BASE + len(dvo.OPS)
    shas = {
        ver: DveOpSpec(name=name, opcode=row, uops=lower(spec, ver=ver),
                       rd1_en=False).sha(ver)
        for ver in ("v3", "v4")
    }
    op = dvo.DveOp(name, spec, subdim=False, uops_sha=shas)
    dvo.OPS.append(op)
    dvo.CUSTOM_DVE_SPECS[name] = spec
    dvo._SUB_OPCODE_FOR_NAME[name] = row
    return op


def _build_nc():
    from concourse import bacc, mybir
    from concourse.tile import TileContext

    exp_op = _register_dve_op()
    f16, f32 = mybir.dt.float16, mybir.dt.float32
    Act = mybir.ActivationFunctionType

    nc = bacc.Bacc(None, target_bir_lowering=False)
    xin = nc.declare_dram_parameter("xin", [NCH, 128, 2048], f16, isOutput=False)
    w1 = nc.declare_dram_parameter("w1", [128, 512], f16, isOutput=False)
    w2 = nc.declare_dram_parameter("w2", [128, 16], f16, isOutput=False)
    c0 = nc.declare_dram_parameter("c0", [128, 1], f32, isOutput=False)
    ab = nc.declare_dram_parameter("ab", [128, 1], f32, isOutput=False)
    out = nc.declare_dram_parameter("out", [NCH, 128, 512], f16, isOutput=True)

    act_scale = float(1.0 / FS)

    with TileContext(nc) as tc:
        with (
            tc.tile_pool(name="const", bufs=1) as cpool,
            tc.tile_pool(name="xp", bufs=2) as xpool,
            tc.tile_pool(name="ep", bufs=3) as epool,
            tc.tile_pool(name="ob", bufs=2) as opool,
            tc.tile_pool(name="ps", bufs=2, space="PSUM") as ppool,
        ):
            w1_t = cpool.tile([128, 512], f16, tag="w1")
            nc.sync.dma_start(out=w1_t[:], in_=w1[:])
            w2_t = cpool.tile([128, 16], f16, tag="w2")
            nc.sync.dma_start(out=w2_t[:], in_=w2[:])
            c0_t = cpool.tile([128, 1], f32, tag="c0")
            nc.sync.dma_start(out=c0_t[:], in_=c0[:])
            ab_t = cpool.tile([128, 1], f32, tag="ab")
            nc.sync.dma_start(out=ab_t[:], in_=ab[:])

            xts, ots = {}, {}
            pend = []    # (pt, et, ch, q) awaiting MM2+copy, one sp behind

            def emit_mm2(pt, et, ch, q):
                # k''-slices 0,1 come from the ScalarE exp half (cols
                # [0,1024)); slices 2,3 from the DVE half. Splitting the
                # accumulation lets the first half overlap the DVE exp.
                rhs4 = et[:].rearrange("z (i mm n) -> z i mm n", i=4, mm=4)
                obs = [
                    pt[32 * m:32 * m + 16, 0:128].rearrange(
                        "p (i n) -> p i n", i=1).broadcast_to([16, 2, 128])
                    for m in range(4)
                ]
                for m in range(4):
                    nc.tensor.matmul(
                        obs[m], w2_t[:, 0:16], rhs4[:, 0:2, m, :],
                        start=True, stop=False, tile_position=(0, 32 * m),
                    )
                for m in range(4):
                    nc.tensor.matmul(
                        obs[m], w2_t[:, 0:16], rhs4[:, 2:4, m, :],
                        start=False, stop=True, tile_position=(0, 32 * m),
                    )
                dst = ots[ch][:, q * 128:(q + 1) * 128]
                if q % 2 == 0:
                    nc.scalar.activation(dst, pt[:, 0:128],
                                         mybir.ActivationFunctionType.Copy,
                                         bias=0.0, scale=1.0)
                else:
                    nc.vector.tensor_copy(dst, pt[:, 0:128])
                if q == 3:
                    nc.sync.dma_start(out=out[ch], in_=ots[ch][:])

            for sp in range(NSP):
                ch, q = sp // 4, sp % 4
                if q == 0:
                    xt = xpool.tile([128, 2048], f16, tag="x")
                    xts[ch] = xt
                    nc.sync.dma_start(out=xt[:], in_=xin[ch])
                    ot = opool.tile([128, 512], f16, tag="o")
                    ots[ch] = ot
                pt = ppool.tile([128, 2048], f32, tag="arg")
                if sp == 0:
                    # PE warmup under the first DMA: ~3.5us of back-to-back
                    # 512-col matmuls bring the HAM clock gate to 8/8.
                    for _ in range(9):
                        nc.tensor.matmul(pt[:, 0:512], w1_t[0:32, 0:128],
                                         w1_t[0:32, 0:512], start=True,
                                         stop=True, tile_position=(0, 0))
                rhs = xts[ch][:, q * 512:(q + 1) * 512]
                for m in range(4):
                    nc.tensor.matmul(
                        pt[:, m * 512:(m + 1) * 512],
                        w1_t[m * 32:(m + 1) * 32, 0:128],
                        rhs[m * 32:(m + 1) * 32, :],
                        start=True, stop=True, tile_position=(m * 32, 0),
                    )
                et = epool.tile([128, 2048], f16, tag="e")
                nc.scalar.activation(
                    et[:, 0:XSPLIT], pt[:, 0:XSPLIT], Act.Exp,
                    bias=ab_t[:], scale=act_scale,
                )
                nc.vector._custom_dve(
                    exp_op, out=et[:, XSPLIT:2048], in0=pt[:, XSPLIT:2048],
                    s0=c0_t[:], s1=float(FB),
                )
                pend.append((pt, et, ch, q))
                if len(pend) > 1:
                    emit_mm2(*pend.pop(0))
            emit_mm2(*pend.pop(0))

    nc.finalize()
    return nc


def _host_pack(x):
    """x: (B, N, K, C) fp32 -> (B, NCH, 128, 2048) fp16 rhs layout.

    partition = 32*m + 4*k_j + ct  (ct in 0..3: x0,x1,x2,||x||^2; m = k'')
    col       = (sp%4)*512 + n_local
    with n = 512*sp + n_local, k = 4*k_j + m.
    """
    xr = x.reshape(B, NSP, 512, 8, 4, C)             # b,sp,nl,kj,m,c
    s = (xr.astype(np.float64) ** 2).sum(-1)         # b,sp,nl,kj,m
    x4 = np.empty((B, NSP, 512, 8, 4, 4), dtype=np.float16)
    x4[..., :C] = xr.astype(np.float16)
    x4[..., C] = s.astype(np.float16)
    # -> (b, sp, m, kj, ct, nl)
    d = x4.transpose(0, 1, 4, 3, 5, 2)
    d = np.ascontiguousarray(d.reshape(B, NSP, 128, 512))
    return np.ascontiguousarray(
        d.reshape(B, NCH, 4, 128, 512).transpose(0, 1, 3, 2, 4)
    ).reshape(B, NCH, 128, 2048)


def _host_weights(kernel_points, sigma):
    kp = np.asarray(kernel_points, dtype=np.float64)
    s2 = float(sigma) ** 2
    bias = -(kp ** 2).sum(-1) / (2.0 * s2)           # (P,)

    w1 = np.zeros((128, 512), dtype=np.float16)
    wv = np.zeros((4, P), dtype=np.float64)
    wv[:C] = (FS * kp / s2).T
    wv[C] = -FS / (2.0 * s2)
    for m in range(4):
        for kj in range(8):
            for ct in range(4):
                w1[32 * m + 4 * kj + ct, 16 * kj:16 * (kj + 1)] = \
                    wv[ct].astype(np.float16)

    w2 = np.zeros((128, 16), dtype=np.float16)
    for kj in range(8):
        for p in range(P):
            w2[16 * kj + p, p] = np.float16(1.0 / K)

    c0 = np.zeros((128, 1), dtype=np.float32)
    ab = np.zeros((128, 1), dtype=np.float32)
    for kj in range(8):
        for p in range(P):
            c0[16 * kj + p, 0] = FA + FS * bias[p]
            ab[16 * kj + p, 0] = bias[p]
    return w1, w2, c0, ab


def _host_unpack(outs):
    """outs: list of 8 per-core (NCH, 128, 512) fp16 -> (B, N, P) fp32."""
    res = np.empty((B, N, P), dtype=np.float32)
    for b, o in enumerate(outs):
        # out[ch, 32m+p (p<16), q*128+n'] = res[n = (4ch+q)*512 + 128m + n', p]
        o4 = o.reshape(NCH, 4, 32, 4, 128)           # ch, m, part32, q, n'
        r = o4[:, :, :P, :, :].transpose(0, 3, 1, 4, 2)  # ch,q,m,n',p
        res[b] = r.reshape(N, P).astype(np.float32)
    return res


def _run(inputs, trace=False, tmpdir=None, trace_cores=None):
    from concourse.bass_utils import run_bass_kernel_spmd

    x = np.asarray(inputs["neighborhoods"], dtype=np.float32)
    d = _host_pack(x)
    w1, w2, c0, ab = _host_weights(inputs["kernel_points"], inputs["sigma"])

    if "nc" not in _CACHE:
        _CACHE["nc"] = _build_nc()
    nc = _CACHE["nc"]

    core_ids = list(range(B))
    in_maps = [
        {"xin": d[b], "w1": w1, "w2": w2, "c0": c0, "ab": ab}
        for b in range(B)
    ]
    res = run_bass_kernel_spmd(nc, in_maps, core_ids, trace=trace,
                               tmpdir=tmpdir, trace_cores=trace_cores)
    return _host_unpack([res.results[b]["out"] for b in range(B)]), res


def kernel(neighborhoods, kernel_points, sigma):
    out, _ = _run({"neighborhoods": neighborhoods,
                   "kernel_points": kernel_points, "sigma": sigma})
    return out


# revision 9
# speedup vs baseline: 1.3999x; 1.1256x over previous
"""Trainium2 Bass kernel for GaussianKernelConv.

Math: out[b,n,p] = mean_k exp(-||x[b,n,k,:] - kp[p,:]||^2 / (2 sigma^2))

Per-core dataflow (one batch b of N=8192, K=32, P=16):
  MM1 (TensorE): t = FS * v_mm, where v_mm = x.(kp_p/s^2) - ||x||^2/(2 s^2).
    Output partitions = (k_j in 8, p in 16). Four row-group matmuls
    (tile_position (32m,0), contract 32 = (k_j, [x0,x1,x2,||x||^2])) per
    512-n superpass; row group m computes the k''=m slice (k = 4*k_j + m)
    for all 512 n, into psum bank m ([128, 512] cols = n_local).
  exp (split):
    ScalarE: exact exp via activation(Exp, scale=1/FS, bias=bias_p) on psum
      cols [0, XSPLIT).
    VectorE: custom 8-stage DVE op  [ (t + C0_p)^2 + FB ]^32  ~= e^(v_mm+bias_p)
      on cols [XSPLIT, 2048)  (max rel err ~9e-3 on the relevant range,
      monotone-tiny below; C0_p = FA + FS*bias_p per partition). Only ~30%
      of each output's K terms take this path, diluting its error ~3x.
  MM2 (TensorE): K-reduction. Contract the 128 (k_j,p) partitions with a
    [128,16] p-selector (1/K). Four column-tiled matmuls (tile_position
    (0,32m), one per n-quarter) each stream 512 cols with a stride-0
    broadcast out AP that revisits the same 128 psum cols 4x -- the k''
    fold happens via PSUM has_written accumulation inside one matmul.
    Result [(m,p'), n'] lands in the just-freed psum bank 0, cast
    fp16 -> SBUF (VectorE) -> DMA out. A 12-matmul warmup burst under the
    first DMA brings the PE HAM clock gate to 8/8 before the pipeline.
  PE emission is software-pipelined: MM1(sp+1) is emitted before MM2(sp) so
  the TensorE never head-of-line blocks on the exp engines.

Sharding: data-parallel over batch B=8 -> 8 cores, one batch each.
"""

import sys

for _p in ("/opt/trn_rl_repo",):
    if _p not in sys.path:
        sys.path.insert(0, _p)

import numpy as np

B, N, K, C, P = 8, 8192, 32, 3, 16
NSP = 16          # superpasses per core; each covers 512 n
NCH = 4           # DMA chunks (epochs); 4 superpasses each
XSPLIT = 1024     # psum cols [0,XSPLIT) -> ScalarE exp; rest -> DVE custom op

# exp approximation constants: [ (FS*w + FA)^2 + FB ]^32 ~= e^w on w in [-12, 0]
FS = 0.02010519997941581
FA = 0.7677708409964104
FB = 0.41026898832429365

_CACHE = {}


def _register_dve_op():
    """Register the 8-stage [quad]^32 exp op with concourse's custom-DVE table."""
    from concourse import dve_ops as dvo
    from concourse.dve_spec import Spec, Src0, C0, C1, sq, lower
    from concourse.dve_uop import DveOpSpec

    name = "EXP_POW32_ANT"
    if name in dvo._SUB_OPCODE_FOR_NAME:
        for op in dvo.OPS:
            if op.name == name:
                return op

    def _ref(in0, in1, s0, s1, imm2):
        t = in0.astype(np.float32) + np.asarray(s0, np.float32).reshape(-1, 1)
        u = t * t + np.float32(s1)
        for _ in range(5):
            u = u * u
        return u

    body = sq(Src0 + C0) + C1
    for _ in range(5):
        body = sq(body)
    spec = Spec(body=body, reference=_ref)
    row = dvo._CUSTOM_DVE_ROW_BASE + len(dvo.OPS)
    shas = {
        ver: DveOpSpec(name=name, opcode=row, uops=lower(spec, ver=ver),
                       rd1_en=False).sha(ver)
        for ver in ("v3", "v4")
    }
    op = dvo.DveOp(name, spec, subdim=False, uops_sha=shas)
    dvo.OPS.append(op)
    dvo.CUSTOM_DVE_SPECS[name] = spec
    dvo._SUB_OPCODE_FOR_NAME[name] = row
    return op


def _build_nc():
    from concourse import bacc, mybir
    from concourse.tile import TileContext

    exp_op = _register_dve_op()
    f16, f32 = mybir.dt.float16, mybir.dt.float32
    Act = mybir.ActivationFunctionType

    nc = bacc.Bacc(None, target_bir_lowering=False)
    xin = nc.declare_dram_parameter("xin", [NCH, 128, 2048], f16, isOutput=False)
    w1 = nc.declare_dram_parameter("w1", [128, 512], f16, isOutput=False)
    w2 = nc.declare_dram_parameter("w2", [128, 16], f16, isOutput=False)
    c0 = nc.declare_dram_parameter("c0", [128, 1], f32, isOutput=False)
    ab = nc.declare_dram_parameter("ab", [128, 1], f32, isOutput=False)
    out = nc.declare_dram_parameter("out", [NCH, 128, 512], f16, isOutput=True)

    act_scale = float(1.0 / FS)

    with TileContext(nc) as tc:
        with (
            tc.tile_pool(name="const", bufs=1) as cpool,
            tc.tile_pool(name="xp", bufs=2) as xpool,
            tc.tile_pool(name="ep", bufs=3) as epool,
            tc.tile_pool(name="ob", bufs=2) as opool,
            tc.tile_pool(name="ps", bufs=2, space="PSUM") as ppool,
        ):
            w1_t = cpool.tile([128, 512], f16, tag="w1")
            nc.sync.dma_start(out=w1_t[:], in_=w1[:])
            w2_t = cpool.tile([128, 16], f16, tag="w2")
            nc.sync.dma_start(out=w2_t[:], in_=w2[:])
            c0_t = cpool.tile([128, 1], f32, tag="c0")
            nc.sync.dma_start(out=c0_t[:], in_=c0[:])
            ab_t = cpool.tile([128, 1], f32, tag="ab")
            nc.sync.dma_start(out=ab_t[:], in_=ab[:])

            xts, ots = {}, {}
            pend = []    # (pt, et, ch, q) awaiting MM2+copy, one sp behind

            def emit_mm2(pta, eta, etd, ch, q):
                # k''-slices 0,1 come from the ScalarE exp half (pt_a/et_a);
                # slices 2,3 from the DVE half (pt_b/et_d). Splitting the
                # accumulation lets the first half overlap the DVE exp.
                ra = eta[:].rearrange("z (i mm n) -> z i mm n", i=2, mm=4)
                rd = etd[:].rearrange("z (i mm n) -> z i mm n", i=2, mm=4)
                obs = [
                    pta[32 * m:32 * m + 16, 0:128].rearrange(
                        "p (i n) -> p i n", i=1).broadcast_to([16, 2, 128])
                    for m in range(4)
                ]
                for m in range(4):
                    nc.tensor.matmul(
                        obs[m], w2_t[:, 0:16], ra[:, :, m, :],
                        start=True, stop=False, tile_position=(0, 32 * m),
                    )
                for m in range(4):
                    nc.tensor.matmul(
                        obs[m], w2_t[:, 0:16], rd[:, :, m, :],
                        start=False, stop=True, tile_position=(0, 32 * m),
                    )
                dst = ots[ch][:, q * 128:(q + 1) * 128]
                if q % 2 == 0:
                    nc.scalar.activation(dst, pta[:, 0:128],
                                         mybir.ActivationFunctionType.Copy,
                                         bias=0.0, scale=1.0)
                else:
                    nc.vector.tensor_copy(dst, pta[:, 0:128])
                if q == 3:
                    nc.sync.dma_start(out=out[ch], in_=ots[ch][:])

            for sp in range(NSP):
                ch, q = sp // 4, sp % 4
                if q == 0:
                    xt = xpool.tile([128, 2048], f16, tag="x")
                    xts[ch] = xt
                    nc.sync.dma_start(out=xt[:], in_=xin[ch])
                    ot = opool.tile([128, 512], f16, tag="o")
                    ots[ch] = ot
                pta = ppool.tile([128, 1024], f32, tag="pa")
                ptb = ppool.tile([128, 1024], f32, tag="pb")
                if sp == 0:
                    # PE warmup under the first DMA: ~3.5us of back-to-back
                    # 512-col matmuls bring the HAM clock gate to 8/8.
                    for _ in range(9):
                        nc.tensor.matmul(pta[:, 0:512], w1_t[0:32, 0:128],
                                         w1_t[0:32, 0:512], start=True,
                                         stop=True, tile_position=(0, 0))
                rhs = xts[ch][:, q * 512:(q + 1) * 512]
                for m in range(4):
                    dst = (pta if m < 2 else ptb)[:, (m % 2) * 512:(m % 2 + 1) * 512]
                    nc.tensor.matmul(
                        dst,
                        w1_t[m * 32:(m + 1) * 32, 0:128],
                        rhs[m * 32:(m + 1) * 32, :],
                        start=True, stop=True, tile_position=(m * 32, 0),
                    )
                eta = epool.tile([128, 1024], f16, tag="ea")
                nc.scalar.activation(
                    eta[:], pta[:], Act.Exp,
                    bias=ab_t[:], scale=act_scale,
                )
                etd = epool.tile([128, 1024], f16, tag="ed")
                nc.vector._custom_dve(
                    exp_op, out=etd[:], in0=ptb[:],
                    s0=c0_t[:], s1=float(FB),
                )
                pend.append((pta, eta, etd, ch, q))
                if len(pend) > 1:
                    emit_mm2(*pend.pop(0))
            emit_mm2(*pend.pop(0))

    nc.finalize()
    return nc


def _host_pack(x):
    """x: (B, N, K, C) fp32 -> (B, NCH, 128, 2048) fp16 rhs layout.

    partition = 32*m + 4*k_j + ct  (ct in 0..3: x0,x1,x2,||x||^2; m = k'')
    col       = (sp%4)*512 + n_local
    with n = 512*sp + n_local, k = 4*k_j + m.
    """
    xr = x.reshape(B, NSP, 512, 8, 4, C)             # b,sp,nl,kj,m,c
    s = (xr.astype(np.float64) ** 2).sum(-1)         # b,sp,nl,kj,m
    x4 = np.empty((B, NSP, 512, 8, 4, 4), dtype=np.float16)
    x4[..., :C] = xr.astype(np.float16)
    x4[..., C] = s.astype(np.float16)
    # -> (b, sp, m, kj, ct, nl)
    d = x4.transpose(0, 1, 4, 3, 5, 2)
    d = np.ascontiguousarray(d.reshape(B, NSP, 128, 512))
    return np.ascontiguousarray(
        d.reshape(B, NCH, 4, 128, 512).transpose(0, 1, 3, 2, 4)
    ).reshape(B, NCH, 128, 2048)


def _host_weights(kernel_points, sigma):
    kp = np.asarray(kernel_points, dtype=np.float64)
    s2 = float(sigma) ** 2
    bias = -(kp ** 2).sum(-1) / (2.0 * s2)           # (P,)

    w1 = np.zeros((128, 512), dtype=np.float16)
    wv = np.zeros((4, P), dtype=np.float64)
    wv[:C] = (FS * kp / s2).T
    wv[C] = -FS / (2.0 * s2)
    for m in range(4):
        for kj in range(8):
            for ct in range(4):
                w1[32 * m + 4 * kj + ct, 16 * kj:16 * (kj + 1)] = \
                    wv[ct].astype(np.float16)

    w2 = np.zeros((128, 16), dtype=np.float16)
    for kj in range(8):
        for p in range(P):
            w2[16 * kj + p, p] = np.float16(1.0 / K)

    c0 = np.zeros((128, 1), dtype=np.float32)
    ab = np.zeros((128, 1), dtype=np.float32)
    for kj in range(8):
        for p in range(P):
            c0[16 * kj + p, 0] = FA + FS * bias[p]
            ab[16 * kj + p, 0] = bias[p]
    return w1, w2, c0, ab


def _host_unpack(outs):
    """outs: list of 8 per-core (NCH, 128, 512) fp16 -> (B, N, P) fp32."""
    res = np.empty((B, N, P), dtype=np.float32)
    for b, o in enumerate(outs):
        # out[ch, 32m+p (p<16), q*128+n'] = res[n = (4ch+q)*512 + 128m + n', p]
        o4 = o.reshape(NCH, 4, 32, 4, 128)           # ch, m, part32, q, n'
        r = o4[:, :, :P, :, :].transpose(0, 3, 1, 4, 2)  # ch,q,m,n',p
        res[b] = r.reshape(N, P).astype(np.float32)
    return res


def _run(inputs, trace=False, tmpdir=None, trace_cores=None):
    from concourse.bass_utils import run_bass_kernel_spmd

    x = np.asarray(inputs["neighborhoods"], dtype=np.float32)
    d = _host_pack(x)
    w1, w2, c0, ab = _host_weights(inputs["kernel_points"], inputs["sigma"])

    if "nc" not in _CACHE:
        _CACHE["nc"] = _build_nc()
    nc = _CACHE["nc"]

    core_ids = list(range(B))
    in_maps = [
        {"xin": d[b], "w1": w1, "w2": w2, "c0": c0, "ab": ab}
        for b in range(B)
    ]
    res = run_bass_kernel_spmd(nc, in_maps, core_ids, trace=trace,
                               tmpdir=tmpdir, trace_cores=trace_cores)
    return _host_unpack([res.results[b]["out"] for b in range(B)]), res


def kernel(neighborhoods, kernel_points, sigma):
    out, _ = _run({"neighborhoods": neighborhoods,
                   "kernel_points": kernel_points, "sigma": sigma})
    return out


# revision 10
# speedup vs baseline: 1.4009x; 1.0007x over previous
"""Trainium2 Bass kernel for GaussianKernelConv.

Math: out[b,n,p] = mean_k exp(-||x[b,n,k,:] - kp[p,:]||^2 / (2 sigma^2))

Per-core dataflow (one batch b of N=8192, K=32, P=16):
  MM1 (TensorE): t = FS * v_mm, where v_mm = x.(kp_p/s^2) - ||x||^2/(2 s^2).
    Output partitions = (k_j in 8, p in 16). Four row-group matmuls
    (tile_position (32m,0), contract 32 = (k_j, [x0,x1,x2,||x||^2])) per
    512-n superpass; row group m computes the k''=m slice (k = 4*k_j + m)
    for all 512 n, into psum bank m ([128, 512] cols = n_local).
  exp (split):
    ScalarE: exact exp via activation(Exp, scale=1/FS, bias=bias_p) on psum
      cols [0, XSPLIT).
    VectorE: custom 8-stage DVE op  [ (t + C0_p)^2 + FB ]^32  ~= e^(v_mm+bias_p)
      on cols [XSPLIT, 2048)  (max rel err ~9e-3 on the relevant range,
      monotone-tiny below; C0_p = FA + FS*bias_p per partition). Only ~30%
      of each output's K terms take this path, diluting its error ~3x.
  MM2 (TensorE): K-reduction. Contract the 128 (k_j,p) partitions with a
    [128,16] p-selector (1/K). Four column-tiled matmuls (tile_position
    (0,32m), one per n-quarter) each stream 512 cols with a stride-0
    broadcast out AP that revisits the same 128 psum cols 4x -- the k''
    fold happens via PSUM has_written accumulation inside one matmul.
    Result [(m,p'), n'] lands in the just-freed psum bank 0, cast
    fp16 -> SBUF (VectorE) -> DMA out. A 12-matmul warmup burst under the
    first DMA brings the PE HAM clock gate to 8/8 before the pipeline.
  PE emission is software-pipelined: MM1(sp+1) is emitted before MM2(sp) so
  the TensorE never head-of-line blocks on the exp engines.

Sharding: data-parallel over batch B=8 -> 8 cores, one batch each.
"""

import sys

for _p in ("/opt/trn_rl_repo",):
    if _p not in sys.path:
        sys.path.insert(0, _p)

import numpy as np

B, N, K, C, P = 8, 8192, 32, 3, 16
NSP = 16          # superpasses per core; each covers 512 n
NCH = 4           # DMA chunks (epochs); 4 superpasses each
XSPLIT = 1024     # psum cols [0,XSPLIT) -> ScalarE exp; rest -> DVE custom op

# exp approximation constants: [ (FS*w + FA)^2 + FB ]^32 ~= e^w on w in [-12, 0]
FS = 0.02010519997941581
FA = 0.7677708409964104
FB = 0.41026898832429365

_CACHE = {}


def _register_dve_op():
    """Register the 8-stage [quad]^32 exp op with concourse's custom-DVE table."""
    from concourse import dve_ops as dvo
    from concourse.dve_spec import Spec, Src0, C0, C1, sq, lower
    from concourse.dve_uop import DveOpSpec

    name = "EXP_POW32_ANT"
    if name in dvo._SUB_OPCODE_FOR_NAME:
        for op in dvo.OPS:
            if op.name == name:
                return op

    def _ref(in0, in1, s0, s1, imm2):
        t = in0.astype(np.float32) + np.asarray(s0, np.float32).reshape(-1, 1)
        u = t * t + np.float32(s1)
        for _ in range(5):
            u = u * u
        return u

    body = sq(Src0 + C0) + C1
    for _ in range(5):
        body = sq(body)
    spec = Spec(body=body, reference=_ref)
    row = dvo._CUSTOM_DVE_ROW_BASE + len(dvo.OPS)
    shas = {
        ver: DveOpSpec(name=name, opcode=row, uops=lower(spec, ver=ver),
                       rd1_en=False).sha(ver)
        for ver in ("v3", "v4")
    }
    op = dvo.DveOp(name, spec, subdim=False, uops_sha=shas)
    dvo.OPS.append(op)
    dvo.CUSTOM_DVE_SPECS[name] = spec
    dvo._SUB_OPCODE_FOR_NAME[name] = row
    return op


def _build_nc():
    from concourse import bacc, mybir
    from concourse.tile import TileContext

    exp_op = _register_dve_op()
    f16, f32 = mybir.dt.float16, mybir.dt.float32
    Act = mybir.ActivationFunctionType

    nc = bacc.Bacc(None, target_bir_lowering=False)
    xin = nc.declare_dram_parameter("xin", [NCH, 128, 2048], f16, isOutput=False)
    w1 = nc.declare_dram_parameter("w1", [128, 512], f16, isOutput=False)
    w2 = nc.declare_dram_parameter("w2", [128, 16], f16, isOutput=False)
    c0 = nc.declare_dram_parameter("c0", [128, 1], f32, isOutput=False)
    ab = nc.declare_dram_parameter("ab", [128, 1], f32, isOutput=False)
    out = nc.declare_dram_parameter("out", [NCH, 128, 512], f16, isOutput=True)

    act_scale = float(1.0 / FS)

    with TileContext(nc) as tc:
        with (
            tc.tile_pool(name="const", bufs=1) as cpool,
            tc.tile_pool(name="xp", bufs=2) as xpool,
            tc.tile_pool(name="ep", bufs=3) as epool,
            tc.tile_pool(name="ob", bufs=2) as opool,
            tc.tile_pool(name="ps", bufs=2, space="PSUM") as ppool,
        ):
            w1_t = cpool.tile([128, 512], f16, tag="w1")
            nc.sync.dma_start(out=w1_t[:], in_=w1[:])
            w2_t = cpool.tile([128, 16], f16, tag="w2")
            nc.sync.dma_start(out=w2_t[:], in_=w2[:])
            c0_t = cpool.tile([128, 1], f32, tag="c0")
            nc.sync.dma_start(out=c0_t[:], in_=c0[:])
            ab_t = cpool.tile([128, 1], f32, tag="ab")
            nc.sync.dma_start(out=ab_t[:], in_=ab[:])

            xts, ots = {}, {}
            pend = []    # (pt, et, ch, q) awaiting MM2+copy, one sp behind

            def emit_mm2(pta, eta, etd, ch, q):
                # k''-slices 0,1 come from the ScalarE exp half (pt_a/et_a);
                # slices 2,3 from the DVE half (pt_b/et_d). Splitting the
                # accumulation lets the first half overlap the DVE exp.
                ra = eta[:].rearrange("z (i mm n) -> z i mm n", i=2, mm=4)
                rd = etd[:].rearrange("z (i mm n) -> z i mm n", i=2, mm=4)
                obs = [
                    pta[32 * m:32 * m + 16, 0:128].rearrange(
                        "p (i n) -> p i n", i=1).broadcast_to([16, 2, 128])
                    for m in range(4)
                ]
                for m in range(4):
                    nc.tensor.matmul(
                        obs[m], w2_t[:, 0:16], ra[:, :, m, :],
                        start=True, stop=False, tile_position=(0, 32 * m),
                    )
                for m in range(4):
                    nc.tensor.matmul(
                        obs[m], w2_t[:, 0:16], rd[:, :, m, :],
                        start=False, stop=True, tile_position=(0, 32 * m),
                    )
                dst = ots[ch][:, q * 128:(q + 1) * 128]
                if q % 2 == 0:
                    nc.scalar.activation(dst, pta[:, 0:128],
                                         mybir.ActivationFunctionType.Copy,
                                         bias=0.0, scale=1.0)
                else:
                    nc.vector.tensor_copy(dst, pta[:, 0:128])
                if q == 3:
                    nc.sync.dma_start(out=out[ch], in_=ots[ch][:])

            for sp in range(NSP):
                ch, q = sp // 4, sp % 4
                if len(pend) > 1:
                    emit_mm2(*pend.pop(0))
                if q == 0:
                    xt = xpool.tile([128, 2048], f16, tag="x")
                    xts[ch] = xt
                    nc.sync.dma_start(out=xt[:], in_=xin[ch])
                    ot = opool.tile([128, 512], f16, tag="o")
                    ots[ch] = ot
                pta = ppool.tile([128, 1024], f32, tag="pa")
                ptb = ppool.tile([128, 1024], f32, tag="pb")
                if sp == 0:
                    # PE warmup under the first DMA: ~3.5us of back-to-back
                    # 512-col matmuls bring the HAM clock gate to 8/8.
                    for _ in range(9):
                        nc.tensor.matmul(pta[:, 0:512], w1_t[0:32, 0:128],
                                         w1_t[0:32, 0:512], start=True,
                                         stop=True, tile_position=(0, 0))
                rhs = xts[ch][:, q * 512:(q + 1) * 512]
                for m in range(4):
                    dst = (pta if m < 2 else ptb)[:, (m % 2) * 512:(m % 2 + 1) * 512]
                    nc.tensor.matmul(
                        dst,
                        w1_t[m * 32:(m + 1) * 32, 0:128],
                        rhs[m * 32:(m + 1) * 32, :],
                        start=True, stop=True, tile_position=(m * 32, 0),
                    )
                eta = epool.tile([128, 1024], f16, tag="ea")
                nc.scalar.activation(
                    eta[:], pta[:], Act.Exp,
                    bias=ab_t[:], scale=act_scale,
                )
                etd = epool.tile([128, 1024], f16, tag="ed")
                nc.vector._custom_dve(
                    exp_op, out=etd[:], in0=ptb[:],
                    s0=c0_t[:], s1=float(FB),
                )
                pend.append((pta, eta, etd, ch, q))
            while pend:
                emit_mm2(*pend.pop(0))

    nc.finalize()
    return nc


def _host_pack(x):
    """x: (B, N, K, C) fp32 -> (B, NCH, 128, 2048) fp16 rhs layout.

    partition = 32*m + 4*k_j + ct  (ct in 0..3: x0,x1,x2,||x||^2; m = k'')
    col       = (sp%4)*512 + n_local
    with n = 512*sp + n_local, k = 4*k_j + m.
    """
    xr = x.reshape(B, NSP, 512, 8, 4, C)             # b,sp,nl,kj,m,c
    s = (xr.astype(np.float64) ** 2).sum(-1)         # b,sp,nl,kj,m
    x4 = np.empty((B, NSP, 512, 8, 4, 4), dtype=np.float16)
    x4[..., :C] = xr.astype(np.float16)
    x4[..., C] = s.astype(np.float16)
    # -> (b, sp, m, kj, ct, nl)
    d = x4.transpose(0, 1, 4, 3, 5, 2)
    d = np.ascontiguousarray(d.reshape(B, NSP, 128, 512))
    return np.ascontiguousarray(
        d.reshape(B, NCH, 4, 128, 512).transpose(0, 1, 3, 2, 4)
    ).reshape(B, NCH, 128, 2048)


def _host_weights(kernel_points, sigma):
    kp = np.asarray(kernel_points, dtype=np.float64)
    s2 = float(sigma) ** 2
    bias = -(kp ** 2).sum(-1) / (2.0 * s2)           # (P,)

    w1 = np.zeros((128, 512), dtype=np.float16)
    wv = np.zeros((4, P), dtype=np.float64)
    wv[:C] = (FS * kp / s2).T
    wv[C] = -FS / (2.0 * s2)
    for m in range(4):
        for kj in range(8):
            for ct in range(4):
                w1[32 * m + 4 * kj + ct, 16 * kj:16 * (kj + 1)] = \
                    wv[ct].astype(np.float16)

    w2 = np.zeros((128, 16), dtype=np.float16)
    for kj in range(8):
        for p in range(P):
            w2[16 * kj + p, p] = np.float16(1.0 / K)

    c0 = np.zeros((128, 1), dtype=np.float32)
    ab = np.zeros((128, 1), dtype=np.float32)
    for kj in range(8):
        for p in range(P):
            c0[16 * kj + p, 0] = FA + FS * bias[p]
            ab[16 * kj + p, 0] = bias[p]
    return w1, w2, c0, ab


def _host_unpack(outs):
    """outs: list of 8 per-core (NCH, 128, 512) fp16 -> (B, N, P) fp32."""
    res = np.empty((B, N, P), dtype=np.float32)
    for b, o in enumerate(outs):
        # out[ch, 32m+p (p<16), q*128+n'] = res[n = (4ch+q)*512 + 128m + n', p]
        o4 = o.reshape(NCH, 4, 32, 4, 128)           # ch, m, part32, q, n'
        r = o4[:, :, :P, :, :].transpose(0, 3, 1, 4, 2)  # ch,q,m,n',p
        res[b] = r.reshape(N, P).astype(np.float32)
    return res


def _run(inputs, trace=False, tmpdir=None, trace_cores=None):
    from concourse.bass_utils import run_bass_kernel_spmd

    x = np.asarray(inputs["neighborhoods"], dtype=np.float32)
    d = _host_pack(x)
    w1, w2, c0, ab = _host_weights(inputs["kernel_points"], inputs["sigma"])

    if "nc" not in _CACHE:
        _CACHE["nc"] = _build_nc()
    nc = _CACHE["nc"]

    core_ids = list(range(B))
    in_maps = [
        {"xin": d[b], "w1": w1, "w2": w2, "c0": c0, "ab": ab}
        for b in range(B)
    ]
    res = run_bass_kernel_spmd(nc, in_maps, core_ids, trace=trace,
                               tmpdir=tmpdir, trace_cores=trace_cores)
    return _host_unpack([res.results[b]["out"] for b in range(B)]), res


def kernel(neighborhoods, kernel_points, sigma):
    out, _ = _run({"neighborhoods": neighborhoods,
                   "kernel_points": kernel_points, "sigma": sigma})
    return out
